# revision 24
# baseline (speedup 1.0000x reference)
"""Trainium2 Bass kernel for PVT-style spatial-reduction attention.

Reference computation (per batch element b, data-parallel over 8 cores):
  q   = x @ Wq                                     [4096, 256]
  xsr = LN(conv4x4s4(x.reshape(64,64,256)) + srb)  [256, 256]
  k,v = xsr @ Wkv                                  [256, 256] each
  o_h = softmax(q_h k_h^T / sqrt(32)) v_h          8 heads of 32
  y   = concat_h(o_h) @ Wp + bp                    [4096, 256]

v5 design notes (overlap/occupancy rewrite on top of v3):
  - Input DMA order: x pieces first (sync queue), sr_kernel in
    tap-quarters (scalar queue) interleaved so the conv accumulates
    per-quarter as pieces land; transposes+Q emitted per x piece.
  - x^T transpose evacuations fused: one strided DVE copy per PSUM
    group of 4 blocks (16 copies instead of 64).
  - conv stationary operands read straight from x^T via strided APs
    (no xTp gather pass); conv mt0/mt1 accumulate in parallel banks
    across quarters.
  - attention loop software-pipelined over stages s=(chunk, head-half):
    scores(s) interleave with PV+denom(s-1) and y(c-1), so PE never
    waits on exp; PSUM: spt 2x2 + o_ps 2 + d_ps 2 = 8 banks.
  - Score path (x^T, Q, conv, LN, K) in f32r; output path (e, V, Osc,
    Wp) in bf16; softmax denominator via ones-matmul col-packed like
    PV; recip + divide fused into the PSUM evacuation on DVE.
"""

import os
import sys

import numpy as np

sys.path.insert(0, "/opt/trn_rl_repo")
os.environ.setdefault("MYCRO_LOCAL_CACHE", "1")

B, N_TOK, DIM = 8, 4096, 256
NH, HD = 8, 32
SR = 4
GRID = 16              # 64/SR
M_KV = GRID * GRID     # 256
LN_EPS = 1e-3
SCALE = float(HD) ** -0.5
CHUNK = 512            # query-token chunk
NCH = N_TOK // CHUNK   # 8
P = 128

# Schraudolph bf16 exp: bits(e^s) ~= int16(A*s_raw + B), s_raw = raw q.k
EXP_A = 128.0 * 1.4426950408889634 * SCALE
EXP_B = 127.0 * 128.0 - 7.42 + 0.5

# which of the 4 (j, mt) exp tile slots per stage go to DVE
DVE_EXP_N = 0
DVE_EXP = lambda j, mt: (2 * j + mt) >= 4 - DVE_EXP_N

LAST_RESULTS = None    # test.py introspects this for profiling info


def build_program(niter=1, loop_n=None):
    import concourse.bass as bass
    import concourse.tile as tile
    from concourse import bacc, mybir
    from concourse.masks import make_identity

    f32 = mybir.dt.float32
    f32r = mybir.dt.float32r
    bf16 = mybir.dt.bfloat16
    i16 = mybir.dt.int16
    i32 = mybir.dt.int32
    ALU = mybir.AluOpType
    ACT = mybir.ActivationFunctionType
    AX = mybir.AxisListType

    def r(ap):
        return ap.bitcast(f32r)

    nc = bacc.Bacc("TRN2", target_bir_lowering=False, debug=False)

    x_d = nc.dram_tensor("x", (N_TOK, DIM), f32, kind="ExternalInput")
    wq_d = nc.dram_tensor("Wq", (DIM, DIM), f32, kind="ExternalInput")
    wkv_d = nc.dram_tensor("Wkv", (DIM, 2 * DIM), f32, kind="ExternalInput")
    srk_d = nc.dram_tensor("sr_kernel", (SR, SR, DIM, DIM), f32, kind="ExternalInput")
    srb_d = nc.dram_tensor("sr_bias", (DIM,), f32, kind="ExternalInput")
    gam_d = nc.dram_tensor("ln_gamma", (DIM,), f32, kind="ExternalInput")
    bet_d = nc.dram_tensor("ln_beta", (DIM,), f32, kind="ExternalInput")
    wp_d = nc.dram_tensor("Wp", (DIM, DIM), f32, kind="ExternalInput")
    bp_d = nc.dram_tensor("bp", (DIM,), f32, kind="ExternalInput")
    y_d = nc.dram_tensor("y", (N_TOK, DIM), f32, kind="ExternalOutput")

    with tile.TileContext(nc) as tc:
        import contextlib
        loop_ctx = (lambda: tc.For_i(0, loop_n, 1)) if loop_n else contextlib.nullcontext
        for _it in range(niter):
          with loop_ctx():
            with tc.tile_pool(name="persist", bufs=1) as pp:
                # ---- persistent SBUF tensors ----
                xT = pp.tile([P, 2, N_TOK], f32r)        # x^T  feature-major
                QT = pp.tile([P, 2, N_TOK], f32r)        # q^T  feature-major
                Osc = pp.tile([P, 2, N_TOK], bf16)      # (attn out)^T, normalized
                KT = pp.tile([P, 2, M_KV], f32r)         # k^T  feature-major
                Vtm = pp.tile([P, 2, DIM], bf16)        # v    token-major
                wp_b = pp.tile([P, 2, DIM], bf16)
                ones32b = pp.tile([P, 32], bf16)        # denominator stationary
                nc.gpsimd.memset(ones32b[:], 1.0)
                btot_full = pp.tile([P, DIM], f32)      # broadcast bias for y
                warm_sb = pp.tile([1, 2], bf16)         # exp-table prewarm scratch
                ident = pp.tile([P, P], f32)
                make_identity(nc, ident[:])

                with tc.tile_pool(name="wts", bufs=1) as wpool:
                    # ======== input DMAs, ordered for earliest compute ====
                    x_r = x_d.rearrange("(to ti) d -> ti to d", ti=P)
                    x_pieces = {}
                    srk_f = wpool.tile([P, 2, SR * SR, DIM], f32r)
                    srk_r = srk_d.rearrange(
                        "kh kw (c2 c1) co -> c1 c2 (kh kw) co", c1=P).bitcast(f32r)
                    wq_f = wpool.tile([P, 2, DIM], f32r)
                    wq_r2 = wq_d.rearrange("(ko ki) j -> ki ko j", ki=P).bitcast(f32r)

                    # ALL input DMAs on the sync queue in priority order
                    # (cross-queue order is not preserved at the HWDGE):
                    # x0, wq, x1-x3, srk_q0, x4-x7, srk q1-3, wkv, wp, rest.
                    def xpiece(p):
                        xp = wpool.tile([P, 4, DIM], f32, name="x_sb", bufs=4)
                        if p == 0:
                            for hh in range(2):
                                nc.sync.dma_start(
                                    xp[:, 2 * hh:2 * hh + 2, :],
                                    x_r[:, 2 * hh:2 * hh + 2, :])
                        else:
                            nc.sync.dma_start(xp[:, :, :], x_r[:, 4 * p:4 * p + 4, :])
                        x_pieces[p] = xp

                    xpiece(0)
                    nc.sync.dma_start(wq_f[:, :, :], wq_r2[:, :, :])
                    gam_sb = wpool.tile([P, 2], f32)
                    nc.sync.dma_start(gam_sb[:], gam_d.rearrange("(ko ki) -> ki ko", ki=P))
                    bet_f = wpool.tile([P, 2], f32)
                    nc.sync.dma_start(bet_f[:], bet_d.rearrange("(ko ki) -> ki ko", ki=P))
                    bp_row = wpool.tile([1, DIM], f32)
                    nc.sync.dma_start(bp_row[:], bp_d[None, :])
                    srb_row = wpool.tile([1, DIM], f32r)
                    nc.sync.dma_start(srb_row[:], srb_d[None, :].bitcast(f32r))
                    for p in range(1, 8):
                        xpiece(p)
                    for tq in range(0, 16, 4):
                        for k in range(2):
                            nc.sync.dma_start(
                                srk_f[:, k, tq:tq + 4, :], srk_r[:, k, tq:tq + 4, :])

                    wkv_f = wpool.tile([P, 2, 2 * DIM], f32r)
                    wkv_r2 = wkv_d.rearrange("(ko ki) j -> ki ko j", ki=P).bitcast(f32r)
                    nc.sync.dma_start(wkv_f[:, :, :], wkv_r2[:, :, :])

                    wp_f = wpool.tile([P, 2, DIM], f32r)
                    wp_r2 = wp_d.rearrange("(ko ki) j -> ki ko j", ki=P).bitcast(f32r)
                    nc.sync.dma_start(wp_f[:, :, :], wp_r2[:, :, :])
                    for k in range(2):
                        nc.vector.tensor_copy(wp_b[:, k, :], wp_f[:, k, :])

                    bet2 = wpool.tile([P, 2, 2], f32r)
                    for k in range(2):
                        for c2 in range(2):
                            nc.vector.tensor_copy(bet2[:, k, c2:c2 + 1], bet_f[:, k:k + 1])
                    ones_row = wpool.tile([1, P], f32r)
                    ones_row_f = wpool.tile([1, P], f32)
                    nc.gpsimd.memset(ones_row_f[:], 1.0)
                    nc.vector.tensor_copy(ones_row[:], ones_row_f[:])

                    # ======== prologue: transposes+Q per x piece, conv per
                    # srk quarter, then LN + K/V ====
                    with (
                        tc.tile_pool(name="proA", bufs=1) as proA,
                        tc.tile_pool(name="psA", bufs=1, space="PSUM") as psA,
                    ):
                        # xT holds x^T in TAP-MAJOR permuted position
                        # order: position p = (4di+dj)*256 + i*16 + j for
                        # token t = 256i + 64di + 4j + dj. Attention is
                        # position-order invariant; only the y store needs to
                        # un-permute. Conv patches are contiguous [tap, m].
                        xT_v = xT.rearrange(
                            "p k (di dj i j) -> p k di dj i j", di=SR, dj=SR, j=GRID)
                        xT_t = xT.rearrange("p k (tap m) -> p k tap m", tap=SR * SR)
                        conv_ps = [
                            psA.tile([P, DIM], f32, name=f"conv_ps{mt}", bufs=1)
                            for mt in range(2)
                        ]

                        def transp_piece(p):
                            xpr = x_pieces[p].rearrange("p t (k e) -> p t k e", k=2)
                            for gl in range(2):
                                g = 2 * p + gl       # g = conv grid row i
                                # tp_ps laid out [k][di_hi][tok128]
                                tp_ps = psA.tile([P, 2, 2, P], f32, name="tp_ps", bufs=2)
                                for u in range(4):
                                    blk = 4 * g + u      # 64 blocks: (tt, k)
                                    tt, k = blk // 2, blk % 2
                                    nc.tensor.transpose(
                                        tp_ps[:, u % 2, u // 2, :],
                                        xpr[:, tt - 4 * p, k, :],
                                        ident[:])
                                # scatter to tap-major: out walks (di, j, dj),
                                # in walks (di, j, dj) with in-strides
                                # (64, 4, 1) over the merged [k][256] block
                                tpf = tp_ps.rearrange(
                                    "p k dh (dl j dj) -> p k (dh dl) j dj",
                                    dl=2, j=GRID)
                                for k in range(2):
                                    nc.vector.tensor_copy(
                                        xT_v[:, k, :, :, g, :].rearrange(
                                            "p di dj j -> p di j dj"),
                                        tpf[:, k, :, :, :])

                        def q_piece(c):
                            for ko in range(2):
                                qt_ps = psA.tile([P, CHUNK], f32, name="qt_ps", bufs=3)
                                for k in range(2):
                                    nc.tensor.matmul(
                                        qt_ps[:],
                                        r(wq_f[:, k, ko * P:(ko + 1) * P]),
                                        r(xT[:, k, c * CHUNK:(c + 1) * CHUNK]),
                                        start=(k == 0), stop=(k == 1),
                                    )
                                if ko == 0 or c == 7:
                                    nc.scalar.copy(
                                        QT[:, ko, c * CHUNK:(c + 1) * CHUNK], qt_ps[:])
                                else:
                                    nc.vector.tensor_copy(
                                        QT[:, ko, c * CHUNK:(c + 1) * CHUNK], qt_ps[:])

                        def conv_quarter(tq):
                            if tq == 0:
                                # sr_bias primed into the PSUM accumulators
                                for mt in range(2):
                                    nc.tensor.matmul(
                                        conv_ps[mt][:], r(ones_row[:]), r(srb_row[:]),
                                        start=True, stop=False)
                            for tap in range(tq, tq + 4):
                                for k in range(2):
                                    for mt in range(2):
                                        nc.tensor.matmul(
                                            conv_ps[mt][:],
                                            r(xT_t[:, k, tap, mt * P:(mt + 1) * P]),
                                            r(srk_f[:, k, tap, :]),
                                            start=False,
                                            stop=(tap == 15 and k == 1),
                                        )

                        transp_piece(0)
                        # warm the Exp ACT table while ACT is idle; copy,
                        # square and exp all live in the same table set, so
                        # this is the only table load in the kernel
                        nc.scalar.activation(warm_sb[:], ident[0:1, 0:2], ACT.Exp)
                        for p in range(1, 8):
                            transp_piece(p)
                        # Q chunks fill the PE idle while conv chases the
                        # sr_kernel quarter DMAs
                        q_piece(0)
                        q_piece(1)
                        conv_quarter(0)
                        q_piece(2)
                        q_piece(3)
                        conv_quarter(4)
                        q_piece(4)
                        q_piece(5)
                        conv_quarter(8)
                        q_piece(6)
                        conv_quarter(12)
                        q_piece(7)

                        # beta contribution: K-bias is softmax-invariant; V-bias
                        # flows through Wp into a per-output-dim constant.
                        bvT = proA.tile([P, 2, 2], f32r)
                        for ko in range(2):
                            bv_ps = psA.tile([P, 2], f32, name="tp_ps", bufs=2)
                            for k in range(2):
                                nc.tensor.matmul(
                                    bv_ps[:],
                                    r(wkv_f[:, k, DIM + ko * P:DIM + (ko + 1) * P]),
                                    r(bet2[:, k, :]),
                                    start=(k == 0), stop=(k == 1),
                                )
                            nc.vector.tensor_copy(bvT[:, ko, :], bv_ps[:])
                        bt_ps = psA.tile([1, DIM], f32, name="qt_ps", bufs=3)
                        for k in range(2):
                            nc.tensor.matmul(
                                bt_ps[:], r(bvT[:, k, 0:1]), r(wp_f[:, k, :]),
                                start=(k == 0), stop=(k == 1),
                            )
                        btot_row = proA.tile([1, DIM], f32r)
                        nc.vector.tensor_add(btot_row[:], bt_ps[:], bp_row[:])
                        btot_bc_ps = psA.tile([P, DIM], f32, name="tp_ps", bufs=2)
                        nc.tensor.matmul(btot_bc_ps[:], r(ones_row[:]), r(btot_row[:]),
                                         start=True, stop=True)
                        nc.vector.tensor_copy(btot_full[:], btot_bc_ps[:])

                        # ---- LN + K/V ----
                        wkv_r = proA.tile([P, 2, 2 * DIM], f32r)
                        for k in range(2):
                            nc.vector.tensor_scalar_mul(
                                wkv_r[:, k, :], wkv_f[:, k, :], gam_sb[:, k:k + 1])
                        xlnT = proA.tile([P, 2, M_KV], f32r)
                        # mean/var per token row via bn_stats straight from
                        # the conv PSUM (bias already inside)
                        bst = proA.tile([P, 2, 6], f32)
                        agg = proA.tile([P, 2, 2], f32)
                        for mt in range(2):
                            nc.vector.bn_stats(bst[:, mt, :], conv_ps[mt][:])
                            nc.vector.bn_aggr(agg[:, mt, :], bst[:, mt, :])
                        # rstd = rsqrt(var + eps) for both mt at once on DVE
                        # (bit trick + 2 Newton steps): no ACT Sqrt table
                        u = proA.tile([P, 2], f32)
                        nc.vector.tensor_scalar(
                            u[:], agg[:, :, 1], 1.0, LN_EPS,
                            op0=ALU.mult, op1=ALU.add)
                        yh = proA.tile([P, 2], i32)
                        nc.vector.tensor_scalar(
                            yh[:], u[:].bitcast(i32), 1, None,
                            op0=ALU.arith_shift_right)
                        yi = proA.tile([P, 2], i32)
                        nc.vector.tensor_scalar(
                            yi[:], yh[:], -1, 0x5F3759DF,
                            op0=ALU.mult, op1=ALU.add)
                        rstd = yi.bitcast(f32)
                        for _nt in range(2):
                            t0 = proA.tile([P, 2], f32, name="ln_t0", bufs=4)
                            nc.vector.tensor_mul(t0[:], u[:], rstd[:])
                            t1 = proA.tile([P, 2], f32, name="ln_t1", bufs=4)
                            nc.vector.tensor_mul(t1[:], t0[:], rstd[:])
                            t2 = proA.tile([P, 2], f32, name="ln_t2", bufs=4)
                            nc.vector.tensor_scalar(
                                t2[:], t1[:], -0.5, 1.5,
                                op0=ALU.mult, op1=ALU.add)
                            rstd_n = proA.tile([P, 2], f32, name="ln_rs", bufs=4)
                            nc.vector.tensor_mul(rstd_n[:], rstd[:], t2[:])
                            rstd = rstd_n
                        # -mean*rstd per row, then xln = conv*rstd + nmr
                        nmr = proA.tile([P, 2], f32)
                        nc.vector.tensor_mul(nmr[:], agg[:, :, 0], rstd[:])
                        nnmr = proA.tile([P, 2], f32)
                        nc.vector.tensor_scalar(nnmr[:], nmr[:], -1.0, None,
                                                op0=ALU.mult)
                        for mt in range(2):
                            xln = proA.tile([P, DIM], f32, name="ln_out", bufs=2)
                            nc.vector.tensor_scalar(
                                xln[:], conv_ps[mt][:],
                                rstd[:, mt:mt + 1], nnmr[:, mt:mt + 1],
                                op0=ALU.mult, op1=ALU.add)
                            for k in range(2):
                                t_ps = psA.tile([P, P], f32, name="tp_ps", bufs=2)
                                nc.tensor.transpose(t_ps[:], xln[:, k * P:(k + 1) * P], ident[:])
                                nc.vector.tensor_copy(xlnT[:, k, mt * P:(mt + 1) * P], t_ps[:])

                        # K^T feature-major (f32)
                        for ko in range(2):
                            kt_ps = psA.tile([P, M_KV], f32, name="qt_ps", bufs=3)
                            for k in range(2):
                                nc.tensor.matmul(
                                    kt_ps[:],
                                    r(wkv_r[:, k, ko * P:(ko + 1) * P]),
                                    r(xlnT[:, k, :]),
                                    start=(k == 0), stop=(k == 1),
                                )
                            nc.vector.tensor_copy(KT[:, ko, :], kt_ps[:])
                        # V token-major (bf16)
                        for mt in range(2):
                            v_ps = psA.tile([P, DIM], f32, name="tp_ps", bufs=2)
                            for k in range(2):
                                nc.tensor.matmul(
                                    v_ps[:],
                                    r(xlnT[:, k, mt * P:(mt + 1) * P]),
                                    r(wkv_r[:, k, DIM:2 * DIM]),
                                    start=(k == 0), stop=(k == 1),
                                )
                            nc.vector.tensor_copy(Vtm[:, mt, :], v_ps[:])

                # ======== attention + y-proj, software-pipelined over
                # stages s = (chunk c, head-half hp) ====
                with (
                    tc.tile_pool(name="attn_sb", bufs=1) as asb,
                    tc.tile_pool(name="psS", bufs=1, space="PSUM") as psS,
                    tc.tile_pool(name="psO", bufs=1, space="PSUM") as psO,
                    tc.tile_pool(name="psD", bufs=1, space="PSUM") as psD,
                ):
                    # token t = 256i + 64di + 4j + dj at position
                    # p = (4di+dj)*256 + i*16 + j; chunk c holds taps
                    # {2c, 2c+1}; y_sb partition ti=(i_lo,j), to=(dj_lo,i_hi)
                    y_v = y_d.rearrange(
                        "(i di j dj) d -> i di j dj d", di=SR, j=GRID, dj=SR)
                    NSTAGE = 2 * NCH
                    ods = {}    # stage -> (o_ps, d_ps)
                    exps = {}   # (stage, j, mt) -> e tile

                    def scores(s, j):
                        c, hp = s // 2, s % 2
                        for mt in range(2):
                            spt = psS.tile([P, 2 * CHUNK], f32, name="sp", bufs=2)
                            for hi in range(2):
                                hh = 2 * j + hi
                                nc.tensor.matmul(
                                    spt[:, CHUNK * hi:CHUNK * (hi + 1)],
                                    r(KT[32 * hh:32 * hh + 32, hp, mt * P:(mt + 1) * P]),
                                    r(QT[32 * hh:32 * hh + 32, hp, c * CHUNK:(c + 1) * CHUNK]),
                                    start=True, stop=True,
                                    tile_position=(32 * hh, 0),
                                )
                            e = asb.tile([P, 2 * CHUNK], bf16, name="expS", bufs=8)
                            if DVE_EXP(j, mt):
                                nc.vector.tensor_scalar(
                                    e[:].bitcast(i16), spt[:],
                                    EXP_A, EXP_B, op0=ALU.mult, op1=ALU.add)
                            else:
                                nc.scalar.activation(
                                    e[:], spt[:], ACT.Exp, scale=SCALE)
                            exps[(s, j, mt)] = e

                    def pvden(s, j):
                        c, hp = s // 2, s % 2
                        if j == 0:
                            ods[s] = (
                                psO.tile([P, CHUNK], f32, name="o_ps", bufs=2),
                                psD.tile([P, CHUNK], f32, name="d_ps", bufs=2),
                            )
                        o_ps, d_ps = ods[s]
                        for mt in range(2):
                            for hi in range(2):
                                hh = 2 * j + hi
                                h = 4 * hp + hh
                                e_ap = exps[(s, j, mt)][:, CHUNK * hi:CHUNK * (hi + 1)]
                                nc.tensor.matmul(
                                    o_ps[32 * hh:32 * hh + 32, :],
                                    Vtm[:, mt, 32 * h:32 * h + 32],
                                    e_ap,
                                    start=(mt == 0), stop=(mt == 1),
                                    tile_position=(0, 32 * hh),
                                )
                                nc.tensor.matmul(
                                    d_ps[32 * hh:32 * hh + 32, :],
                                    ones32b[:],
                                    e_ap,
                                    start=(mt == 0), stop=(mt == 1),
                                    tile_position=(0, 32 * hh),
                                )

                    def norm_evac(s):
                        c, hp = s // 2, s % 2
                        o_ps, d_ps = ods.pop(s)
                        dr = asb.tile([P, CHUNK], f32, name="dr", bufs=2)
                        nc.vector.reciprocal_approx_fast(dr[:], d_ps[:])
                        nc.vector.tensor_mul(
                            Osc[:, hp, c * CHUNK:(c + 1) * CHUNK], o_ps[:], dr[:]
                        )
                        for jj in range(2):
                            for mt in range(2):
                                del exps[(s, jj, mt)]

                    def yproj(c):
                        y_sb = asb.tile([P, 4, DIM], f32, name="y_sb", bufs=2)
                        for half in range(2):
                            # share the d_ps ring banks (freed by norm_evac)
                            y_ps = psD.tile([P, CHUNK], f32, name="d_ps", bufs=2)
                            for tl in range(2):
                                tt = 4 * c + 2 * half + tl
                                for k in range(2):
                                    nc.tensor.matmul(
                                        y_ps[:, tl * DIM:(tl + 1) * DIM],
                                        Osc[:, k, tt * P:(tt + 1) * P],
                                        wp_b[:, k, :],
                                        start=(k == 0), stop=(k == 1),
                                    )
                            for tl in range(2):
                                nc.vector.scalar_tensor_tensor(
                                    y_sb[:, 2 * half + tl, :],
                                    y_ps[:, tl * DIM:(tl + 1) * DIM], 0.0,
                                    btot_full[:],
                                    op0=ALU.bypass, op1=ALU.add,
                                )
                        for to in range(4):
                            tap = 2 * c + to // 2
                            di, dj = tap // SR, tap % SR
                            ih = to % 2
                            # SBUF side stays [128, 256]; the balancer splits
                            # partitions against the [8, 16, 256] DRAM AP
                            nc.sync.dma_start(
                                y_v[ih * 8:ih * 8 + 8, di, :, dj, :],
                                y_sb[:, to, :])

                    # scores run one stage ahead of PV/denom so exp
                    # latency is always covered
                    scores(0, 0)
                    scores(0, 1)
                    for s in range(NSTAGE):
                        if s + 1 < NSTAGE:
                            scores(s + 1, 0)
                        pvden(s, 0)
                        if s + 1 < NSTAGE:
                            scores(s + 1, 1)
                        pvden(s, 1)
                        norm_evac(s)
                        if s % 2 == 1:
                            yproj(s // 2)

    return nc


def kernel(**inputs):
    global LAST_RESULTS
    from concourse.bass_utils import run_bass_kernel_spmd

    f = lambda a: np.ascontiguousarray(np.asarray(a, dtype=np.float32))
    x = f(inputs["x"])
    shared = {
        k: f(inputs[k])
        for k in ("Wq", "Wkv", "sr_kernel", "sr_bias", "ln_gamma", "ln_beta", "Wp", "bp")
    }
    nc = build_program()
    if not nc.is_finalized():
        nc.finalize()
    in_maps = [dict(x=x[b], **shared) for b in range(B)]
    res = run_bass_kernel_spmd(
        nc, in_maps, core_ids=list(range(B)),
        trace=bool(int(os.environ.get("KERNEL_TRACE", "0"))),
    )
    LAST_RESULTS = res
    return np.stack([r["y"] for r in res.results], axis=0)


# revision 26
# speedup vs baseline: 1.3665x; 1.3665x over previous
"""Trainium2 Bass kernel for PVT-style spatial-reduction attention.

Reference computation (per batch element b, data-parallel over 8 cores):
  q   = x @ Wq                                     [4096, 256]
  xsr = LN(conv4x4s4(x.reshape(64,64,256)) + srb)  [256, 256]
  k,v = xsr @ Wkv                                  [256, 256] each
  o_h = softmax(q_h k_h^T / sqrt(32)) v_h          8 heads of 32
  y   = concat_h(o_h) @ Wp + bp                    [4096, 256]

v5 design notes (overlap/occupancy rewrite on top of v3):
  - Input DMA order: x pieces first (sync queue), sr_kernel in
    tap-quarters (scalar queue) interleaved so the conv accumulates
    per-quarter as pieces land; transposes+Q emitted per x piece.
  - x^T transpose evacuations fused: one strided DVE copy per PSUM
    group of 4 blocks (16 copies instead of 64).
  - conv stationary operands read straight from x^T via strided APs
    (no xTp gather pass); conv mt0/mt1 accumulate in parallel banks
    across quarters.
  - attention loop software-pipelined over stages s=(chunk, head-half):
    scores(s) interleave with PV+denom(s-1) and y(c-1), so PE never
    waits on exp; PSUM: spt 2x2 + o_ps 2 + d_ps 2 = 8 banks.
  - Score path (x^T, Q, conv, LN, K) in f32r; output path (e, V, Osc,
    Wp) in bf16; softmax denominator via ones-matmul col-packed like
    PV; recip + divide fused into the PSUM evacuation on DVE.
"""

import os
import sys

import numpy as np

sys.path.insert(0, "/opt/trn_rl_repo")
os.environ.setdefault("MYCRO_LOCAL_CACHE", "1")

B, N_TOK, DIM = 8, 4096, 256
NH, HD = 8, 32
SR = 4
GRID = 16              # 64/SR
M_KV = GRID * GRID     # 256
LN_EPS = 1e-3
SCALE = float(HD) ** -0.5
CHUNK = 512            # query-token chunk
NCH = N_TOK // CHUNK   # 8
P = 128

# Schraudolph bf16 exp: bits(e^s) ~= int16(A*s_raw + B), s_raw = raw q.k
EXP_A = 128.0 * 1.4426950408889634 * SCALE
EXP_B = 127.0 * 128.0 - 7.42 + 0.5

# which of the 4 (j, mt) exp tile slots per stage go to DVE
DVE_EXP_N = 0
DVE_EXP = lambda j, mt: (2 * j + mt) >= 4 - DVE_EXP_N

LAST_RESULTS = None    # test.py introspects this for profiling info


def build_program(niter=1, loop_n=None):
    import concourse.bass as bass
    import concourse.tile as tile
    from concourse import bacc, mybir
    from concourse.masks import make_identity

    f32 = mybir.dt.float32
    f32r = mybir.dt.float32r
    bf16 = mybir.dt.bfloat16
    i16 = mybir.dt.int16
    i32 = mybir.dt.int32
    ALU = mybir.AluOpType
    ACT = mybir.ActivationFunctionType
    AX = mybir.AxisListType

    def r(ap):
        return ap.bitcast(f32r)

    nc = bacc.Bacc("TRN2", target_bir_lowering=False, debug=False)

    x_d = nc.dram_tensor("x", (N_TOK, DIM), f32, kind="ExternalInput")
    wq_d = nc.dram_tensor("Wq", (DIM, DIM), f32, kind="ExternalInput")
    wkv_d = nc.dram_tensor("Wkv", (DIM, 2 * DIM), f32, kind="ExternalInput")
    srk_d = nc.dram_tensor("sr_kernel", (SR, SR, DIM, DIM), f32, kind="ExternalInput")
    srb_d = nc.dram_tensor("sr_bias", (DIM,), f32, kind="ExternalInput")
    gam_d = nc.dram_tensor("ln_gamma", (DIM,), f32, kind="ExternalInput")
    bet_d = nc.dram_tensor("ln_beta", (DIM,), f32, kind="ExternalInput")
    wp_d = nc.dram_tensor("Wp", (DIM, DIM), f32, kind="ExternalInput")
    bp_d = nc.dram_tensor("bp", (DIM,), f32, kind="ExternalInput")
    y_d = nc.dram_tensor("y", (N_TOK, DIM), f32, kind="ExternalOutput")

    with tile.TileContext(nc) as tc:
        import contextlib
        loop_ctx = (lambda: tc.For_i(0, loop_n, 1)) if loop_n else contextlib.nullcontext
        for _it in range(niter):
          with loop_ctx():
            with tc.tile_pool(name="persist", bufs=1) as pp:
                # ---- persistent SBUF tensors ----
                xT = pp.tile([P, 2, N_TOK], f32r)        # x^T  feature-major
                QT = pp.tile([P, 2, N_TOK], f32r)        # q^T  feature-major
                Osc = pp.tile([P, 2, N_TOK], bf16)      # (attn out)^T, normalized
                KT = pp.tile([P, 2, M_KV], f32r)         # k^T  feature-major
                Vtm = pp.tile([P, 2, DIM], bf16)        # v    token-major
                wp_b = pp.tile([P, 2, DIM], bf16)
                ones32b = pp.tile([P, 32], bf16)        # denominator stationary
                nc.gpsimd.memset(ones32b[:], 1.0)
                btot_full = pp.tile([P, DIM], f32)      # broadcast bias for y
                warm_sb = pp.tile([1, 2], bf16)         # exp-table prewarm scratch
                ident = pp.tile([P, P], f32)
                make_identity(nc, ident[:])

                with tc.tile_pool(name="wts", bufs=1) as wpool:
                    # ======== input DMAs, ordered for earliest compute ====
                    x_r = x_d.rearrange("(to ti) d -> ti to d", ti=P)
                    x_pieces = {}
                    srk_f = wpool.tile([P, 2, SR * SR, DIM], f32r)
                    srk_r = srk_d.rearrange(
                        "kh kw (c2 c1) co -> c1 c2 (kh kw) co", c1=P).bitcast(f32r)
                    wq_f = wpool.tile([P, 2, DIM], f32r)
                    wq_r2 = wq_d.rearrange("(ko ki) j -> ki ko j", ki=P).bitcast(f32r)

                    # ALL input DMAs on the sync queue in priority order
                    # (cross-queue order is not preserved at the HWDGE):
                    # x0, wq, x1-x3, srk_q0, x4-x7, srk q1-3, wkv, wp, rest.
                    def xpiece(p):
                        xp = wpool.tile([P, 4, DIM], f32, name="x_sb", bufs=4)
                        if p == 0:
                            for hh in range(2):
                                nc.sync.dma_start(
                                    xp[:, 2 * hh:2 * hh + 2, :],
                                    x_r[:, 2 * hh:2 * hh + 2, :])
                        else:
                            nc.sync.dma_start(xp[:, :, :], x_r[:, 4 * p:4 * p + 4, :])
                        x_pieces[p] = xp

                    xpiece(0)
                    nc.sync.dma_start(wq_f[:, :, :], wq_r2[:, :, :])
                    gam_sb = wpool.tile([P, 2], f32)
                    nc.sync.dma_start(gam_sb[:], gam_d.rearrange("(ko ki) -> ki ko", ki=P))
                    bet_f = wpool.tile([P, 2], f32)
                    nc.sync.dma_start(bet_f[:], bet_d.rearrange("(ko ki) -> ki ko", ki=P))
                    bp_row = wpool.tile([1, DIM], f32)
                    nc.sync.dma_start(bp_row[:], bp_d[None, :])
                    srb_row = wpool.tile([1, DIM], f32r)
                    nc.sync.dma_start(srb_row[:], srb_d[None, :].bitcast(f32r))
                    for p in range(1, 8):
                        xpiece(p)
                    for tq in range(0, 16, 2):
                        for k in range(2):
                            nc.sync.dma_start(
                                srk_f[:, k, tq:tq + 2, :], srk_r[:, k, tq:tq + 2, :])

                    wkv_f = wpool.tile([P, 2, 2 * DIM], f32r)
                    wkv_r2 = wkv_d.rearrange("(ko ki) j -> ki ko j", ki=P).bitcast(f32r)
                    nc.sync.dma_start(wkv_f[:, :, :], wkv_r2[:, :, :])

                    wp_f = wpool.tile([P, 2, DIM], f32r)
                    wp_r2 = wp_d.rearrange("(ko ki) j -> ki ko j", ki=P).bitcast(f32r)
                    nc.sync.dma_start(wp_f[:, :, :], wp_r2[:, :, :])
                    for k in range(2):
                        nc.vector.tensor_copy(wp_b[:, k, :], wp_f[:, k, :])

                    bet2 = wpool.tile([P, 2, 2], f32r)
                    for k in range(2):
                        for c2 in range(2):
                            nc.vector.tensor_copy(bet2[:, k, c2:c2 + 1], bet_f[:, k:k + 1])
                    ones_row = wpool.tile([1, P], f32r)
                    ones_row_f = wpool.tile([1, P], f32)
                    nc.gpsimd.memset(ones_row_f[:], 1.0)
                    nc.vector.tensor_copy(ones_row[:], ones_row_f[:])

                    # ======== prologue: transposes+Q per x piece, conv per
                    # srk quarter, then LN + K/V ====
                    with (
                        tc.tile_pool(name="proA", bufs=1) as proA,
                        tc.tile_pool(name="psA", bufs=1, space="PSUM") as psA,
                    ):
                        # xT holds x^T in TAP-MAJOR permuted position
                        # order: position p = (4di+dj)*256 + i*16 + j for
                        # token t = 256i + 64di + 4j + dj. Attention is
                        # position-order invariant; only the y store needs to
                        # un-permute. Conv patches are contiguous [tap, m].
                        xT_v = xT.rearrange(
                            "p k (di dj i j) -> p k di dj i j", di=SR, dj=SR, j=GRID)
                        xT_t = xT.rearrange("p k (tap m) -> p k tap m", tap=SR * SR)
                        conv_ps = [
                            psA.tile([P, DIM], f32, name=f"conv_ps{mt}", bufs=1)
                            for mt in range(2)
                        ]

                        def transp_piece(p):
                            xpr = x_pieces[p].rearrange("p t (k e) -> p t k e", k=2)
                            for gl in range(2):
                                g = 2 * p + gl       # g = conv grid row i
                                # tp_ps laid out [k][di_hi][tok128]
                                tp_ps = psA.tile([P, 2, 2, P], f32, name="tp_ps", bufs=2)
                                for u in range(4):
                                    blk = 4 * g + u      # 64 blocks: (tt, k)
                                    tt, k = blk // 2, blk % 2
                                    nc.tensor.transpose(
                                        tp_ps[:, u % 2, u // 2, :],
                                        xpr[:, tt - 4 * p, k, :],
                                        ident[:])
                                # scatter to tap-major: out walks (di, j, dj),
                                # in walks (di, j, dj) with in-strides
                                # (64, 4, 1) over the merged [k][256] block
                                tpf = tp_ps.rearrange(
                                    "p k dh (dl j dj) -> p k (dh dl) j dj",
                                    dl=2, j=GRID)
                                for k in range(2):
                                    nc.vector.tensor_copy(
                                        xT_v[:, k, :, :, g, :].rearrange(
                                            "p di dj j -> p di j dj"),
                                        tpf[:, k, :, :, :])

                        def q_piece(c):
                            for ko in range(2):
                                qt_ps = psA.tile([P, CHUNK], f32, name="qt_ps", bufs=3)
                                for k in range(2):
                                    nc.tensor.matmul(
                                        qt_ps[:],
                                        r(wq_f[:, k, ko * P:(ko + 1) * P]),
                                        r(xT[:, k, c * CHUNK:(c + 1) * CHUNK]),
                                        start=(k == 0), stop=(k == 1),
                                    )
                                if ko == 0 or c == 7:
                                    nc.scalar.copy(
                                        QT[:, ko, c * CHUNK:(c + 1) * CHUNK], qt_ps[:])
                                else:
                                    nc.vector.tensor_copy(
                                        QT[:, ko, c * CHUNK:(c + 1) * CHUNK], qt_ps[:])

                        def conv_quarter(tq):
                            if tq == 0:
                                # sr_bias primed into the PSUM accumulators
                                for mt in range(2):
                                    nc.tensor.matmul(
                                        conv_ps[mt][:], r(ones_row[:]), r(srb_row[:]),
                                        start=True, stop=False)
                            for tap in range(tq, tq + 4):
                                for k in range(2):
                                    for mt in range(2):
                                        nc.tensor.matmul(
                                            conv_ps[mt][:],
                                            r(xT_t[:, k, tap, mt * P:(mt + 1) * P]),
                                            r(srk_f[:, k, tap, :]),
                                            start=False,
                                            stop=(tap == 15 and k == 1),
                                        )

                        transp_piece(0)
                        # warm the Exp ACT table while ACT is idle; copy,
                        # square and exp all live in the same table set, so
                        # this is the only table load in the kernel
                        nc.scalar.activation(warm_sb[:], ident[0:1, 0:2], ACT.Exp)
                        for p in range(1, 8):
                            transp_piece(p)
                        # Q chunks fill the PE idle while conv chases the
                        # sr_kernel quarter DMAs
                        q_piece(0)
                        q_piece(1)
                        conv_quarter(0)
                        q_piece(2)
                        q_piece(3)
                        conv_quarter(4)
                        q_piece(4)
                        q_piece(5)
                        conv_quarter(8)
                        q_piece(6)
                        conv_quarter(12)
                        q_piece(7)

                        # beta contribution: K-bias is softmax-invariant; V-bias
                        # flows through Wp into a per-output-dim constant.
                        bvT = proA.tile([P, 2, 2], f32r)
                        for ko in range(2):
                            bv_ps = psA.tile([P, 2], f32, name="tp_ps", bufs=2)
                            for k in range(2):
                                nc.tensor.matmul(
                                    bv_ps[:],
                                    r(wkv_f[:, k, DIM + ko * P:DIM + (ko + 1) * P]),
                                    r(bet2[:, k, :]),
                                    start=(k == 0), stop=(k == 1),
                                )
                            nc.vector.tensor_copy(bvT[:, ko, :], bv_ps[:])
                        bt_ps = psA.tile([1, DIM], f32, name="qt_ps", bufs=3)
                        for k in range(2):
                            nc.tensor.matmul(
                                bt_ps[:], r(bvT[:, k, 0:1]), r(wp_f[:, k, :]),
                                start=(k == 0), stop=(k == 1),
                            )
                        btot_row = proA.tile([1, DIM], f32r)
                        nc.vector.tensor_add(btot_row[:], bt_ps[:], bp_row[:])
                        btot_bc_ps = psA.tile([P, DIM], f32, name="tp_ps", bufs=2)
                        nc.tensor.matmul(btot_bc_ps[:], r(ones_row[:]), r(btot_row[:]),
                                         start=True, stop=True)
                        nc.vector.tensor_copy(btot_full[:], btot_bc_ps[:])

                        # ---- LN + K/V ----
                        wkv_r = proA.tile([P, 2, 2 * DIM], f32r)
                        for k in range(2):
                            nc.vector.tensor_scalar_mul(
                                wkv_r[:, k, :], wkv_f[:, k, :], gam_sb[:, k:k + 1])
                        xlnT = proA.tile([P, 2, M_KV], f32r)
                        # mean/var per token row via bn_stats straight from
                        # the conv PSUM (bias already inside)
                        bst = proA.tile([P, 2, 6], f32)
                        agg = proA.tile([P, 2, 2], f32)
                        for mt in range(2):
                            nc.vector.bn_stats(bst[:, mt, :], conv_ps[mt][:])
                            nc.vector.bn_aggr(agg[:, mt, :], bst[:, mt, :])
                        # rstd = rsqrt(var + eps) for both mt at once on DVE
                        # (bit trick + 2 Newton steps): no ACT Sqrt table
                        u = proA.tile([P, 2], f32)
                        nc.vector.tensor_scalar(
                            u[:], agg[:, :, 1], 1.0, LN_EPS,
                            op0=ALU.mult, op1=ALU.add)
                        yh = proA.tile([P, 2], i32)
                        nc.vector.tensor_scalar(
                            yh[:], u[:].bitcast(i32), 1, None,
                            op0=ALU.arith_shift_right)
                        yi = proA.tile([P, 2], i32)
                        nc.vector.tensor_scalar(
                            yi[:], yh[:], -1, 0x5F3759DF,
                            op0=ALU.mult, op1=ALU.add)
                        rstd = yi.bitcast(f32)
                        for _nt in range(2):
                            t0 = proA.tile([P, 2], f32, name="ln_t0", bufs=4)
                            nc.vector.tensor_mul(t0[:], u[:], rstd[:])
                            t1 = proA.tile([P, 2], f32, name="ln_t1", bufs=4)
                            nc.vector.tensor_mul(t1[:], t0[:], rstd[:])
                            t2 = proA.tile([P, 2], f32, name="ln_t2", bufs=4)
                            nc.vector.tensor_scalar(
                                t2[:], t1[:], -0.5, 1.5,
                                op0=ALU.mult, op1=ALU.add)
                            rstd_n = proA.tile([P, 2], f32, name="ln_rs", bufs=4)
                            nc.vector.tensor_mul(rstd_n[:], rstd[:], t2[:])
                            rstd = rstd_n
                        # -mean*rstd per row, then xln = conv*rstd + nmr
                        nmr = proA.tile([P, 2], f32)
                        nc.vector.tensor_mul(nmr[:], agg[:, :, 0], rstd[:])
                        nnmr = proA.tile([P, 2], f32)
                        nc.vector.tensor_scalar(nnmr[:], nmr[:], -1.0, None,
                                                op0=ALU.mult)
                        for mt in range(2):
                            xln = proA.tile([P, DIM], f32, name="ln_out", bufs=2)
                            nc.vector.tensor_scalar(
                                xln[:], conv_ps[mt][:],
                                rstd[:, mt:mt + 1], nnmr[:, mt:mt + 1],
                                op0=ALU.mult, op1=ALU.add)
                            for k in range(2):
                                t_ps = psA.tile([P, P], f32, name="tp_ps", bufs=2)
                                nc.tensor.transpose(t_ps[:], xln[:, k * P:(k + 1) * P], ident[:])
                                nc.vector.tensor_copy(xlnT[:, k, mt * P:(mt + 1) * P], t_ps[:])

                        # K^T feature-major (f32)
                        for ko in range(2):
                            kt_ps = psA.tile([P, M_KV], f32, name="qt_ps", bufs=3)
                            for k in range(2):
                                nc.tensor.matmul(
                                    kt_ps[:],
                                    r(wkv_r[:, k, ko * P:(ko + 1) * P]),
                                    r(xlnT[:, k, :]),
                                    start=(k == 0), stop=(k == 1),
                                )
                            nc.vector.tensor_copy(KT[:, ko, :], kt_ps[:])
                        # V token-major (bf16)
                        for mt in range(2):
                            v_ps = psA.tile([P, DIM], f32, name="tp_ps", bufs=2)
                            for k in range(2):
                                nc.tensor.matmul(
                                    v_ps[:],
                                    r(xlnT[:, k, mt * P:(mt + 1) * P]),
                                    r(wkv_r[:, k, DIM:2 * DIM]),
                                    start=(k == 0), stop=(k == 1),
                                )
                            nc.vector.tensor_copy(Vtm[:, mt, :], v_ps[:])

                # ======== attention + y-proj, software-pipelined over
                # stages s = (chunk c, head-half hp) ====
                with (
                    tc.tile_pool(name="attn_sb", bufs=1) as asb,
                    tc.tile_pool(name="psS", bufs=1, space="PSUM") as psS,
                    tc.tile_pool(name="psO", bufs=1, space="PSUM") as psO,
                    tc.tile_pool(name="psD", bufs=1, space="PSUM") as psD,
                ):
                    # token t = 256i + 64di + 4j + dj at position
                    # p = (4di+dj)*256 + i*16 + j; chunk c holds taps
                    # {2c, 2c+1}; y_sb partition ti=(i_lo,j), to=(dj_lo,i_hi)
                    y_v = y_d.rearrange(
                        "(i di j dj) d -> i di j dj d", di=SR, j=GRID, dj=SR)
                    NSTAGE = 2 * NCH
                    ods = {}    # stage -> (o_ps, d_ps)
                    exps = {}   # (stage, j, mt) -> e tile

                    def scores(s, j):
                        c, hp = s // 2, s % 2
                        for mt in range(2):
                            spt = psS.tile([P, 2 * CHUNK], f32, name="sp", bufs=2)
                            for hi in range(2):
                                hh = 2 * j + hi
                                nc.tensor.matmul(
                                    spt[:, CHUNK * hi:CHUNK * (hi + 1)],
                                    r(KT[32 * hh:32 * hh + 32, hp, mt * P:(mt + 1) * P]),
                                    r(QT[32 * hh:32 * hh + 32, hp, c * CHUNK:(c + 1) * CHUNK]),
                                    start=True, stop=True,
                                    tile_position=(32 * hh, 0),
                                )
                            e = asb.tile([P, 2 * CHUNK], bf16, name="expS", bufs=8)
                            if DVE_EXP(j, mt):
                                nc.vector.tensor_scalar(
                                    e[:].bitcast(i16), spt[:],
                                    EXP_A, EXP_B, op0=ALU.mult, op1=ALU.add)
                            else:
                                nc.scalar.activation(
                                    e[:], spt[:], ACT.Exp, scale=SCALE)
                            exps[(s, j, mt)] = e

                    def pvden(s, j):
                        c, hp = s // 2, s % 2
                        if j == 0:
                            ods[s] = (
                                psO.tile([P, CHUNK], f32, name="o_ps", bufs=2),
                                psD.tile([P, CHUNK], f32, name="d_ps", bufs=2),
                            )
                        o_ps, d_ps = ods[s]
                        for hi in range(2):
                            hh = 2 * j + hi
                            h = 4 * hp + hh
                            for mt in range(2):
                                e_ap = exps[(s, j, mt)][:, CHUNK * hi:CHUNK * (hi + 1)]
                                nc.tensor.matmul(
                                    o_ps[32 * hh:32 * hh + 32, :],
                                    Vtm[:, mt, 32 * h:32 * h + 32],
                                    e_ap,
                                    start=(mt == 0), stop=(mt == 1),
                                    tile_position=(0, 32 * hh),
                                )
                            for mt in range(2):
                                e_ap = exps[(s, j, mt)][:, CHUNK * hi:CHUNK * (hi + 1)]
                                nc.tensor.matmul(
                                    d_ps[32 * hh:32 * hh + 32, :],
                                    ones32b[:],
                                    e_ap,
                                    start=(mt == 0), stop=(mt == 1),
                                    tile_position=(0, 32 * hh),
                                )

                    def norm_evac(s):
                        c, hp = s // 2, s % 2
                        o_ps, d_ps = ods.pop(s)
                        dr = asb.tile([P, CHUNK], f32, name="dr", bufs=2)
                        nc.vector.reciprocal_approx_fast(dr[:], d_ps[:])
                        nc.vector.tensor_mul(
                            Osc[:, hp, c * CHUNK:(c + 1) * CHUNK], o_ps[:], dr[:]
                        )
                        for jj in range(2):
                            for mt in range(2):
                                del exps[(s, jj, mt)]

                    def yproj(c):
                        y_sb = asb.tile([P, 4, DIM], f32, name="y_sb", bufs=2)
                        for half in range(2):
                            # share the d_ps ring banks (freed by norm_evac)
                            y_ps = psD.tile([P, CHUNK], f32, name="d_ps", bufs=2)
                            for tl in range(2):
                                tt = 4 * c + 2 * half + tl
                                for k in range(2):
                                    nc.tensor.matmul(
                                        y_ps[:, tl * DIM:(tl + 1) * DIM],
                                        Osc[:, k, tt * P:(tt + 1) * P],
                                        wp_b[:, k, :],
                                        start=(k == 0), stop=(k == 1),
                                    )
                            for tl in range(2):
                                nc.vector.scalar_tensor_tensor(
                                    y_sb[:, 2 * half + tl, :],
                                    y_ps[:, tl * DIM:(tl + 1) * DIM], 0.0,
                                    btot_full[:],
                                    op0=ALU.bypass, op1=ALU.add,
                                )
                        for to in range(4):
                            tap = 2 * c + to // 2
                            di, dj = tap // SR, tap % SR
                            ih = to % 2
                            # SBUF side stays [128, 256]; the balancer splits
                            # partitions against the [8, 16, 256] DRAM AP
                            nc.sync.dma_start(
                                y_v[ih * 8:ih * 8 + 8, di, :, dj, :],
                                y_sb[:, to, :])

                    # scores run one stage ahead of PV/denom so exp
                    # latency is always covered
                    scores(0, 0)
                    scores(0, 1)
                    for s in range(NSTAGE):
                        if s + 1 < NSTAGE:
                            scores(s + 1, 0)
                        pvden(s, 0)
                        if s + 1 < NSTAGE:
                            scores(s + 1, 1)
                        pvden(s, 1)
                        norm_evac(s)
                        if s % 2 == 1:
                            yproj(s // 2)

    return nc


def kernel(**inputs):
    global LAST_RESULTS
    from concourse.bass_utils import run_bass_kernel_spmd

    f = lambda a: np.ascontiguousarray(np.asarray(a, dtype=np.float32))
    x = f(inputs["x"])
    shared = {
        k: f(inputs[k])
        for k in ("Wq", "Wkv", "sr_kernel", "sr_bias", "ln_gamma", "ln_beta", "Wp", "bp")
    }
    nc = build_program()
    if not nc.is_finalized():
        nc.finalize()
    in_maps = [dict(x=x[b], **shared) for b in range(B)]
    res = run_bass_kernel_spmd(
        nc, in_maps, core_ids=list(range(B)),
        trace=bool(int(os.environ.get("KERNEL_TRACE", "0"))),
    )
    LAST_RESULTS = res
    return np.stack([r["y"] for r in res.results], axis=0)


# revision 27
# speedup vs baseline: 1.4209x; 1.0398x over previous
"""Trainium2 Bass kernel for PVT-style spatial-reduction attention.

Reference computation (per batch element b, data-parallel over 8 cores):
  q   = x @ Wq                                     [4096, 256]
  xsr = LN(conv4x4s4(x.reshape(64,64,256)) + srb)  [256, 256]
  k,v = xsr @ Wkv                                  [256, 256] each
  o_h = softmax(q_h k_h^T / sqrt(32)) v_h          8 heads of 32
  y   = concat_h(o_h) @ Wp + bp                    [4096, 256]

v5 design notes (overlap/occupancy rewrite on top of v3):
  - Input DMA order: x pieces first (sync queue), sr_kernel in
    tap-quarters (scalar queue) interleaved so the conv accumulates
    per-quarter as pieces land; transposes+Q emitted per x piece.
  - x^T transpose evacuations fused: one strided DVE copy per PSUM
    group of 4 blocks (16 copies instead of 64).
  - conv stationary operands read straight from x^T via strided APs
    (no xTp gather pass); conv mt0/mt1 accumulate in parallel banks
    across quarters.
  - attention loop software-pipelined over stages s=(chunk, head-half):
    scores(s) interleave with PV+denom(s-1) and y(c-1), so PE never
    waits on exp; PSUM: spt 2x2 + o_ps 2 + d_ps 2 = 8 banks.
  - Score path (x^T, Q, conv, LN, K) in f32r; output path (e, V, Osc,
    Wp) in bf16; softmax denominator via ones-matmul col-packed like
    PV; recip + divide fused into the PSUM evacuation on DVE.
"""

import os
import sys

import numpy as np

sys.path.insert(0, "/opt/trn_rl_repo")
os.environ.setdefault("MYCRO_LOCAL_CACHE", "1")

B, N_TOK, DIM = 8, 4096, 256
NH, HD = 8, 32
SR = 4
GRID = 16              # 64/SR
M_KV = GRID * GRID     # 256
LN_EPS = 1e-3
SCALE = float(HD) ** -0.5
CHUNK = 512            # query-token chunk
NCH = N_TOK // CHUNK   # 8
P = 128

# Schraudolph bf16 exp: bits(e^s) ~= int16(A*s_raw + B), s_raw = raw q.k
EXP_A = 128.0 * 1.4426950408889634 * SCALE
EXP_B = 127.0 * 128.0 - 7.42 + 0.5

# which of the 4 (j, mt) exp tile slots per stage go to DVE
DVE_EXP_N = 0
DVE_EXP = lambda j, mt: (2 * j + mt) >= 4 - DVE_EXP_N

LAST_RESULTS = None    # test.py introspects this for profiling info


def build_program(niter=1, loop_n=None):
    import concourse.bass as bass
    import concourse.tile as tile
    from concourse import bacc, mybir
    from concourse.masks import make_identity

    f32 = mybir.dt.float32
    f32r = mybir.dt.float32r
    bf16 = mybir.dt.bfloat16
    i16 = mybir.dt.int16
    i32 = mybir.dt.int32
    ALU = mybir.AluOpType
    ACT = mybir.ActivationFunctionType
    AX = mybir.AxisListType

    def r(ap):
        return ap.bitcast(f32r)

    nc = bacc.Bacc("TRN2", target_bir_lowering=False, debug=False)

    x_d = nc.dram_tensor("x", (N_TOK, DIM), f32, kind="ExternalInput")
    wq_d = nc.dram_tensor("Wq", (DIM, DIM), f32, kind="ExternalInput")
    wkv_d = nc.dram_tensor("Wkv", (DIM, 2 * DIM), f32, kind="ExternalInput")
    srk_d = nc.dram_tensor("sr_kernel", (SR, SR, DIM, DIM), f32, kind="ExternalInput")
    srb_d = nc.dram_tensor("sr_bias", (DIM,), f32, kind="ExternalInput")
    gam_d = nc.dram_tensor("ln_gamma", (DIM,), f32, kind="ExternalInput")
    bet_d = nc.dram_tensor("ln_beta", (DIM,), f32, kind="ExternalInput")
    wp_d = nc.dram_tensor("Wp", (DIM, DIM), f32, kind="ExternalInput")
    bp_d = nc.dram_tensor("bp", (DIM,), f32, kind="ExternalInput")
    y_d = nc.dram_tensor("y", (N_TOK, DIM), f32, kind="ExternalOutput")

    with tile.TileContext(nc) as tc:
        import contextlib
        loop_ctx = (lambda: tc.For_i(0, loop_n, 1)) if loop_n else contextlib.nullcontext
        for _it in range(niter):
          with loop_ctx():
            with tc.tile_pool(name="persist", bufs=1) as pp:
                # ---- persistent SBUF tensors ----
                xT = pp.tile([P, 2, N_TOK], f32r)        # x^T  feature-major
                QT = pp.tile([P, 2, N_TOK], f32r)        # q^T  feature-major
                Osc = pp.tile([P, 2, N_TOK], bf16)      # (attn out)^T, normalized
                KT = pp.tile([P, 2, M_KV], f32r)         # k^T  feature-major
                Vtm = pp.tile([P, 2, DIM], bf16)        # v    token-major
                wp_b = pp.tile([P, 2, DIM], bf16)
                ones32b = pp.tile([P, 32], bf16)        # denominator stationary
                nc.gpsimd.memset(ones32b[:], 1.0)
                btot_full = pp.tile([P, DIM], f32)      # broadcast bias for y
                warm_sb = pp.tile([1, 2], bf16)         # exp-table prewarm scratch
                ident = pp.tile([P, P], f32)
                make_identity(nc, ident[:])

                with tc.tile_pool(name="wts", bufs=1) as wpool:
                    # ======== input DMAs, ordered for earliest compute ====
                    x_r = x_d.rearrange("(to ti) d -> ti to d", ti=P)
                    x_pieces = {}
                    srk_f = wpool.tile([P, 2, SR * SR, DIM], f32r)
                    srk_r = srk_d.rearrange(
                        "kh kw (c2 c1) co -> c1 c2 (kh kw) co", c1=P).bitcast(f32r)
                    wq_f = wpool.tile([P, 2, DIM], f32r)
                    wq_r2 = wq_d.rearrange("(ko ki) j -> ki ko j", ki=P).bitcast(f32r)

                    # ALL input DMAs on the sync queue in priority order
                    # (cross-queue order is not preserved at the HWDGE):
                    # x0, wq, x1-x3, srk_q0, x4-x7, srk q1-3, wkv, wp, rest.
                    def xpiece(p):
                        xp = wpool.tile([P, 4, DIM], f32, name="x_sb", bufs=4)
                        if p == 0:
                            for hh in range(2):
                                nc.sync.dma_start(
                                    xp[:, 2 * hh:2 * hh + 2, :],
                                    x_r[:, 2 * hh:2 * hh + 2, :])
                        else:
                            nc.sync.dma_start(xp[:, :, :], x_r[:, 4 * p:4 * p + 4, :])
                        x_pieces[p] = xp

                    xpiece(0)
                    nc.sync.dma_start(wq_f[:, :, :], wq_r2[:, :, :])
                    gam_sb = wpool.tile([P, 2], f32)
                    nc.sync.dma_start(gam_sb[:], gam_d.rearrange("(ko ki) -> ki ko", ki=P))
                    bet_f = wpool.tile([P, 2], f32)
                    nc.sync.dma_start(bet_f[:], bet_d.rearrange("(ko ki) -> ki ko", ki=P))
                    bp_row = wpool.tile([1, DIM], f32)
                    nc.sync.dma_start(bp_row[:], bp_d[None, :])
                    srb_row = wpool.tile([1, DIM], f32r)
                    nc.sync.dma_start(srb_row[:], srb_d[None, :].bitcast(f32r))
                    for p in range(1, 8):
                        xpiece(p)
                    for tq in range(0, 16, 2):
                        for k in range(2):
                            nc.sync.dma_start(
                                srk_f[:, k, tq:tq + 2, :], srk_r[:, k, tq:tq + 2, :])

                    wkv_f = wpool.tile([P, 2, 2 * DIM], f32r)
                    wkv_r2 = wkv_d.rearrange("(ko ki) j -> ki ko j", ki=P).bitcast(f32r)
                    nc.sync.dma_start(wkv_f[:, :, :], wkv_r2[:, :, :])

                    wp_f = wpool.tile([P, 2, DIM], f32r)
                    wp_r2 = wp_d.rearrange("(ko ki) j -> ki ko j", ki=P).bitcast(f32r)
                    nc.sync.dma_start(wp_f[:, :, :], wp_r2[:, :, :])
                    for k in range(2):
                        nc.vector.tensor_copy(wp_b[:, k, :], wp_f[:, k, :])

                    bet2 = wpool.tile([P, 2, 2], f32r)
                    for k in range(2):
                        for c2 in range(2):
                            nc.vector.tensor_copy(bet2[:, k, c2:c2 + 1], bet_f[:, k:k + 1])
                    ones_row = wpool.tile([1, P], f32r)
                    ones_row_f = wpool.tile([1, P], f32)
                    nc.gpsimd.memset(ones_row_f[:], 1.0)
                    nc.vector.tensor_copy(ones_row[:], ones_row_f[:])

                    # ======== prologue: transposes+Q per x piece, conv per
                    # srk quarter, then LN + K/V ====
                    with (
                        tc.tile_pool(name="proA", bufs=1) as proA,
                        tc.tile_pool(name="psA", bufs=1, space="PSUM") as psA,
                    ):
                        # xT holds x^T in TAP-MAJOR permuted position
                        # order: position p = (4di+dj)*256 + i*16 + j for
                        # token t = 256i + 64di + 4j + dj. Attention is
                        # position-order invariant; only the y store needs to
                        # un-permute. Conv patches are contiguous [tap, m].
                        xT_v = xT.rearrange(
                            "p k (di dj i j) -> p k di dj i j", di=SR, dj=SR, j=GRID)
                        xT_t = xT.rearrange("p k (tap m) -> p k tap m", tap=SR * SR)
                        conv_ps = [
                            psA.tile([P, DIM], f32, name=f"conv_ps{mt}", bufs=1)
                            for mt in range(2)
                        ]

                        def transp_piece(p):
                            xpr = x_pieces[p].rearrange("p t (k e) -> p t k e", k=2)
                            for gl in range(2):
                                g = 2 * p + gl       # g = conv grid row i
                                # tp_ps laid out [k][di_hi][tok128]
                                tp_ps = psA.tile([P, 2, 2, P], f32, name="tp_ps", bufs=2)
                                for u in range(4):
                                    blk = 4 * g + u      # 64 blocks: (tt, k)
                                    tt, k = blk // 2, blk % 2
                                    nc.tensor.transpose(
                                        tp_ps[:, u % 2, u // 2, :],
                                        xpr[:, tt - 4 * p, k, :],
                                        ident[:])
                                # scatter to tap-major: out walks (di, j, dj),
                                # in walks (di, j, dj) with in-strides
                                # (64, 4, 1) over the merged [k][256] block
                                tpf = tp_ps.rearrange(
                                    "p k dh (dl j dj) -> p k (dh dl) j dj",
                                    dl=2, j=GRID)
                                for k in range(2):
                                    nc.vector.tensor_copy(
                                        xT_v[:, k, :, :, g, :].rearrange(
                                            "p di dj j -> p di j dj"),
                                        tpf[:, k, :, :, :])

                        def q_piece(c):
                            for ko in range(2):
                                qt_ps = psA.tile([P, CHUNK], f32, name="qt_ps", bufs=3)
                                for k in range(2):
                                    nc.tensor.matmul(
                                        qt_ps[:],
                                        r(wq_f[:, k, ko * P:(ko + 1) * P]),
                                        r(xT[:, k, c * CHUNK:(c + 1) * CHUNK]),
                                        start=(k == 0), stop=(k == 1),
                                    )
                                if ko == 0 or c == 7:
                                    nc.scalar.copy(
                                        QT[:, ko, c * CHUNK:(c + 1) * CHUNK], qt_ps[:])
                                else:
                                    nc.vector.tensor_copy(
                                        QT[:, ko, c * CHUNK:(c + 1) * CHUNK], qt_ps[:])

                        def conv_quarter(tq):
                            if tq == 0:
                                # sr_bias primed into the PSUM accumulators
                                for mt in range(2):
                                    nc.tensor.matmul(
                                        conv_ps[mt][:], r(ones_row[:]), r(srb_row[:]),
                                        start=True, stop=False)
                            for tap in range(tq, tq + 4):
                                for k in range(2):
                                    for mt in range(2):
                                        nc.tensor.matmul(
                                            conv_ps[mt][:],
                                            r(xT_t[:, k, tap, mt * P:(mt + 1) * P]),
                                            r(srk_f[:, k, tap, :]),
                                            start=False,
                                            stop=(tap == 15 and k == 1),
                                        )

                        transp_piece(0)
                        # warm the Exp ACT table while ACT is idle; copy,
                        # square and exp all live in the same table set, so
                        # this is the only table load in the kernel
                        nc.scalar.activation(warm_sb[:], ident[0:1, 0:2], ACT.Exp)
                        for p in range(1, 8):
                            transp_piece(p)
                        # Q chunks fill the PE idle while conv chases the
                        # sr_kernel quarter DMAs
                        q_piece(0)
                        q_piece(1)
                        conv_quarter(0)
                        q_piece(2)
                        q_piece(3)
                        conv_quarter(4)
                        q_piece(4)
                        q_piece(5)
                        conv_quarter(8)
                        q_piece(6)
                        conv_quarter(12)
                        q_piece(7)

                        # beta contribution: K-bias is softmax-invariant; V-bias
                        # flows through Wp into a per-output-dim constant.
                        bvT = proA.tile([P, 2, 2], f32r)
                        for ko in range(2):
                            bv_ps = psA.tile([P, 2], f32, name="tp_ps", bufs=2)
                            for k in range(2):
                                nc.tensor.matmul(
                                    bv_ps[:],
                                    r(wkv_f[:, k, DIM + ko * P:DIM + (ko + 1) * P]),
                                    r(bet2[:, k, :]),
                                    start=(k == 0), stop=(k == 1),
                                )
                            nc.vector.tensor_copy(bvT[:, ko, :], bv_ps[:])
                        bt_ps = psA.tile([1, DIM], f32, name="qt_ps", bufs=3)
                        for k in range(2):
                            nc.tensor.matmul(
                                bt_ps[:], r(bvT[:, k, 0:1]), r(wp_f[:, k, :]),
                                start=(k == 0), stop=(k == 1),
                            )
                        btot_row = proA.tile([1, DIM], f32r)
                        nc.vector.tensor_add(btot_row[:], bt_ps[:], bp_row[:])
                        btot_bc_ps = psA.tile([P, DIM], f32, name="tp_ps", bufs=2)
                        nc.tensor.matmul(btot_bc_ps[:], r(ones_row[:]), r(btot_row[:]),
                                         start=True, stop=True)
                        nc.vector.tensor_copy(btot_full[:], btot_bc_ps[:])

                        # ---- LN + K/V ----
                        wkv_r = proA.tile([P, 2, 2 * DIM], f32r)
                        for k in range(2):
                            nc.vector.tensor_scalar_mul(
                                wkv_r[:, k, :], wkv_f[:, k, :], gam_sb[:, k:k + 1])
                        xlnT = proA.tile([P, 2, M_KV], f32r)
                        # mean/var per token row via bn_stats straight from
                        # the conv PSUM (bias already inside)
                        bst = proA.tile([P, 2, 6], f32)
                        agg = proA.tile([P, 2, 2], f32)
                        for mt in range(2):
                            nc.vector.bn_stats(bst[:, mt, :], conv_ps[mt][:])
                            nc.vector.bn_aggr(agg[:, mt, :], bst[:, mt, :])
                        # rstd = rsqrt(var + eps) for both mt at once on DVE
                        # (bit trick + 2 Newton steps): no ACT Sqrt table
                        u = proA.tile([P, 2], f32)
                        nc.vector.tensor_scalar(
                            u[:], agg[:, :, 1], 1.0, LN_EPS,
                            op0=ALU.mult, op1=ALU.add)
                        yh = proA.tile([P, 2], i32)
                        nc.vector.tensor_scalar(
                            yh[:], u[:].bitcast(i32), 1, None,
                            op0=ALU.arith_shift_right)
                        yi = proA.tile([P, 2], i32)
                        nc.vector.tensor_scalar(
                            yi[:], yh[:], -1, 0x5F3759DF,
                            op0=ALU.mult, op1=ALU.add)
                        rstd = yi.bitcast(f32)
                        for _nt in range(2):
                            t0 = proA.tile([P, 2], f32, name="ln_t0", bufs=4)
                            nc.vector.tensor_mul(t0[:], u[:], rstd[:])
                            t1 = proA.tile([P, 2], f32, name="ln_t1", bufs=4)
                            nc.vector.tensor_mul(t1[:], t0[:], rstd[:])
                            t2 = proA.tile([P, 2], f32, name="ln_t2", bufs=4)
                            nc.vector.tensor_scalar(
                                t2[:], t1[:], -0.5, 1.5,
                                op0=ALU.mult, op1=ALU.add)
                            rstd_n = proA.tile([P, 2], f32, name="ln_rs", bufs=4)
                            nc.vector.tensor_mul(rstd_n[:], rstd[:], t2[:])
                            rstd = rstd_n
                        # mean*rstd per row, then xln = conv*rstd - mr
                        nmr = proA.tile([P, 2], f32)
                        nc.vector.tensor_mul(nmr[:], agg[:, :, 0], rstd[:])
                        for mt in range(2):
                            xln = proA.tile([P, DIM], f32, name="ln_out", bufs=2)
                            nc.vector.tensor_scalar(
                                xln[:], conv_ps[mt][:],
                                rstd[:, mt:mt + 1], nmr[:, mt:mt + 1],
                                op0=ALU.mult, op1=ALU.subtract)
                            for k in range(2):
                                t_ps = psA.tile([P, P], f32, name="tp_ps", bufs=2)
                                nc.tensor.transpose(t_ps[:], xln[:, k * P:(k + 1) * P], ident[:])
                                nc.vector.tensor_copy(xlnT[:, k, mt * P:(mt + 1) * P], t_ps[:])

                        # K^T feature-major (f32)
                        for ko in range(2):
                            kt_ps = psA.tile([P, M_KV], f32, name="qt_ps", bufs=3)
                            for k in range(2):
                                nc.tensor.matmul(
                                    kt_ps[:],
                                    r(wkv_r[:, k, ko * P:(ko + 1) * P]),
                                    r(xlnT[:, k, :]),
                                    start=(k == 0), stop=(k == 1),
                                )
                            if ko == 0:
                                nc.scalar.copy(KT[:, ko, :], kt_ps[:])
                            else:
                                nc.vector.tensor_copy(KT[:, ko, :], kt_ps[:])
                        # V token-major (bf16)
                        for mt in range(2):
                            v_ps = psA.tile([P, DIM], f32, name="tp_ps", bufs=2)
                            for k in range(2):
                                nc.tensor.matmul(
                                    v_ps[:],
                                    r(xlnT[:, k, mt * P:(mt + 1) * P]),
                                    r(wkv_r[:, k, DIM:2 * DIM]),
                                    start=(k == 0), stop=(k == 1),
                                )
                            nc.vector.tensor_copy(Vtm[:, mt, :], v_ps[:])

                # ======== attention + y-proj, software-pipelined over
                # stages s = (chunk c, head-half hp) ====
                with (
                    tc.tile_pool(name="attn_sb", bufs=1) as asb,
                    tc.tile_pool(name="psS", bufs=1, space="PSUM") as psS,
                    tc.tile_pool(name="psO", bufs=1, space="PSUM") as psO,
                    tc.tile_pool(name="psD", bufs=1, space="PSUM") as psD,
                ):
                    # token t = 256i + 64di + 4j + dj at position
                    # p = (4di+dj)*256 + i*16 + j; chunk c holds taps
                    # {2c, 2c+1}; y_sb partition ti=(i_lo,j), to=(dj_lo,i_hi)
                    y_v = y_d.rearrange(
                        "(i di j dj) d -> i di j dj d", di=SR, j=GRID, dj=SR)
                    NSTAGE = 2 * NCH
                    ods = {}    # stage -> (o_ps, d_ps)
                    exps = {}   # (stage, j, mt) -> e tile

                    def scores(s, j):
                        c, hp = s // 2, s % 2
                        for mt in range(2):
                            spt = psS.tile([P, 2 * CHUNK], f32, name="sp", bufs=2)
                            for hi in range(2):
                                hh = 2 * j + hi
                                nc.tensor.matmul(
                                    spt[:, CHUNK * hi:CHUNK * (hi + 1)],
                                    r(KT[32 * hh:32 * hh + 32, hp, mt * P:(mt + 1) * P]),
                                    r(QT[32 * hh:32 * hh + 32, hp, c * CHUNK:(c + 1) * CHUNK]),
                                    start=True, stop=True,
                                    tile_position=(32 * hh, 0),
                                )
                            e = asb.tile([P, 2 * CHUNK], bf16, name="expS", bufs=8)
                            if DVE_EXP(j, mt):
                                nc.vector.tensor_scalar(
                                    e[:].bitcast(i16), spt[:],
                                    EXP_A, EXP_B, op0=ALU.mult, op1=ALU.add)
                            else:
                                nc.scalar.activation(
                                    e[:], spt[:], ACT.Exp, scale=SCALE)
                            exps[(s, j, mt)] = e

                    def pvden(s, j):
                        c, hp = s // 2, s % 2
                        if j == 0:
                            ods[s] = (
                                psO.tile([P, CHUNK], f32, name="o_ps", bufs=2),
                                psD.tile([P, CHUNK], f32, name="d_ps", bufs=2),
                            )
                        o_ps, d_ps = ods[s]
                        for hi in range(2):
                            hh = 2 * j + hi
                            h = 4 * hp + hh
                            for mt in range(2):
                                e_ap = exps[(s, j, mt)][:, CHUNK * hi:CHUNK * (hi + 1)]
                                nc.tensor.matmul(
                                    o_ps[32 * hh:32 * hh + 32, :],
                                    Vtm[:, mt, 32 * h:32 * h + 32],
                                    e_ap,
                                    start=(mt == 0), stop=(mt == 1),
                                    tile_position=(0, 32 * hh),
                                )
                            for mt in range(2):
                                e_ap = exps[(s, j, mt)][:, CHUNK * hi:CHUNK * (hi + 1)]
                                nc.tensor.matmul(
                                    d_ps[32 * hh:32 * hh + 32, :],
                                    ones32b[:],
                                    e_ap,
                                    start=(mt == 0), stop=(mt == 1),
                                    tile_position=(0, 32 * hh),
                                )

                    def norm_evac(s):
                        c, hp = s // 2, s % 2
                        o_ps, d_ps = ods.pop(s)
                        dr = asb.tile([P, CHUNK], f32, name="dr", bufs=2)
                        nc.vector.reciprocal_approx_fast(dr[:], d_ps[:])
                        nc.vector.tensor_mul(
                            Osc[:, hp, c * CHUNK:(c + 1) * CHUNK], o_ps[:], dr[:]
                        )
                        for jj in range(2):
                            for mt in range(2):
                                del exps[(s, jj, mt)]

                    def yproj(c):
                        y_sb = asb.tile([P, 4, DIM], f32, name="y_sb", bufs=2)
                        for half in range(2):
                            # share the d_ps ring banks (freed by norm_evac)
                            y_ps = psD.tile([P, CHUNK], f32, name="d_ps", bufs=2)
                            for tl in range(2):
                                tt = 4 * c + 2 * half + tl
                                for k in range(2):
                                    nc.tensor.matmul(
                                        y_ps[:, tl * DIM:(tl + 1) * DIM],
                                        Osc[:, k, tt * P:(tt + 1) * P],
                                        wp_b[:, k, :],
                                        start=(k == 0), stop=(k == 1),
                                    )
                            for tl in range(2):
                                nc.vector.scalar_tensor_tensor(
                                    y_sb[:, 2 * half + tl, :],
                                    y_ps[:, tl * DIM:(tl + 1) * DIM], 0.0,
                                    btot_full[:],
                                    op0=ALU.bypass, op1=ALU.add,
                                )
                        for to in range(4):
                            tap = 2 * c + to // 2
                            di, dj = tap // SR, tap % SR
                            ih = to % 2
                            # SBUF side stays [128, 256]; the balancer splits
                            # partitions against the [8, 16, 256] DRAM AP
                            nc.sync.dma_start(
                                y_v[ih * 8:ih * 8 + 8, di, :, dj, :],
                                y_sb[:, to, :])

                    # scores run one stage ahead of PV/denom so exp
                    # latency is always covered
                    scores(0, 0)
                    scores(0, 1)
                    for s in range(NSTAGE):
                        if s + 1 < NSTAGE:
                            scores(s + 1, 0)
                        pvden(s, 0)
                        if s + 1 < NSTAGE:
                            scores(s + 1, 1)
                        pvden(s, 1)
                        norm_evac(s)
                        if s % 2 == 1:
                            yproj(s // 2)

    return nc


def kernel(**inputs):
    global LAST_RESULTS
    from concourse.bass_utils import run_bass_kernel_spmd

    f = lambda a: np.ascontiguousarray(np.asarray(a, dtype=np.float32))
    x = f(inputs["x"])
    shared = {
        k: f(inputs[k])
        for k in ("Wq", "Wkv", "sr_kernel", "sr_bias", "ln_gamma", "ln_beta", "Wp", "bp")
    }
    nc = build_program()
    if not nc.is_finalized():
        nc.finalize()
    in_maps = [dict(x=x[b], **shared) for b in range(B)]
    res = run_bass_kernel_spmd(
        nc, in_maps, core_ids=list(range(B)),
        trace=bool(int(os.environ.get("KERNEL_TRACE", "0"))),
    )
    LAST_RESULTS = res
    return np.stack([r["y"] for r in res.results], axis=0)
